# revision 1
# baseline (speedup 1.0000x reference)
"""Trainium2 Bass kernel: 8-layer ternary (BitNet-1.58) dense transformer.

Model (per reference):
    h = embed[input_ids]                                  # (B=2, S=1024, H=2048)
    8x: y = h @ ternary(W_l)^T + b_l ; h = LN(y + h)*g+b  # H=2048
    h = LN(h)*final_g + final_b
    logits = h @ ternary(head_W)^T                        # (B, S, V=32000)

Sharding over 8 NeuronCores:
  - Layers: data-parallel over the 2048 tokens (256 tokens/core). Each core
    streams the full (bf16 ternary) layer weights; no collectives.
  - Head: 8-way tensor-parallel over vocab (4000 vocab rows/core). The final
    hidden states are exchanged with a single 1MB-per-core AllGather of the
    pre-transposed bf16 activations; each core then computes all 2048 tokens
    x its vocab shard, and the host concatenates along vocab.

Ternary weights are sent as exact {-1,0,+1} bf16 tensors; the per-tensor
fp32 scale is folded into the activation transpose-copy (ScalarE) so matmul
inputs are bf16 while the residual/LN path stays fp32.
"""

import os
import sys

import numpy as np

try:
    import concourse.bass as bass
except ImportError:  # grading container should have it on sys.path already
    sys.path.insert(0, "/opt/trn_rl_repo")
    import concourse.bass as bass

import ml_dtypes
import concourse.mybir as mybir
import concourse.tile as tile
from concourse import bacc
from concourse.bass_utils import run_bass_kernel_spmd
from contextlib import ExitStack

F32 = mybir.dt.float32
BF16 = mybir.dt.bfloat16
AX = mybir.AxisListType
OP = mybir.AluOpType
AF = mybir.ActivationFunctionType
EPS = 1e-5

# Full-size problem config (B=2, S=1024 -> 2048 tokens).
CFG_FULL = dict(L=8, H=2048, NTOK=2048, NC=8, TT=2, VS=4000, QV=1000, NV=500, CH=512)

def build_nc(cfg, scales, head_scale):
    L, H, NTOK, NC, TT = cfg["L"], cfg["H"], cfg["NTOK"], cfg["NC"], cfg["TT"]
    VS, QV, NV, CH = cfg["VS"], cfg["QV"], cfg["NV"], cfg["CH"]
    KT = H // 128
    KH = KT // 2  # k-tiles per weight half
    NCH = H // CH
    assert NTOK == NC * TT * 128 and VS % QV == 0 and QV % NV == 0 and KT % 2 == 0
    NQ = VS // QV
    NVQ = QV // NV

    nc = bacc.Bacc("TRN2", target_bir_lowering=False, debug=False, num_devices=NC)
    h0 = nc.declare_dram_parameter("h0", [TT, 128, H], F32, isOutput=False)
    w_ = nc.declare_dram_parameter("w", [L, KT, 128, H], BF16, isOutput=False)
    lng = nc.declare_dram_parameter("lng", [L, H], BF16, isOutput=False)
    lnb = nc.declare_dram_parameter("lnb", [L, H], BF16, isOutput=False)
    lbias = nc.declare_dram_parameter("lbias", [L, H], BF16, isOutput=False)
    fing = nc.declare_dram_parameter("fing", [H], BF16, isOutput=False)
    finb = nc.declare_dram_parameter("finb", [H], BF16, isOutput=False)
    hw_ = nc.declare_dram_parameter("hw", [KT, 128, VS], BF16, isOutput=False)
    ident_d = nc.declare_dram_parameter("ident", [128, 128], F32, isOutput=False)
    eps_d = nc.declare_dram_parameter("eps", [128, 1], F32, isOutput=False)
    out = nc.declare_dram_parameter("out", [NTOK, VS], F32, isOutput=True)
    hT_loc = nc.dram_tensor("hT_loc", [128, TT, KT, 128], BF16)
    hT_all = nc.dram_tensor(
        "hT_all",
        [NC, 128, TT, KT, 128],
        BF16,
        addr_space="Shared" if NC > 4 else "Local",
    )

    with tile.TileContext(nc) as tc:
        hT_store_insts = []
        with ExitStack() as ctxA:
            consts = ctxA.enter_context(tc.tile_pool(name="consts", bufs=1))
            state = ctxA.enter_context(tc.tile_pool(name="state", bufs=4))
            zpool = ctxA.enter_context(tc.tile_pool(name="z", bufs=2))
            hTp = ctxA.enter_context(tc.tile_pool(name="hT", bufs=2))
            wp = ctxA.enter_context(tc.tile_pool(name="w", bufs=3))
            gbp = ctxA.enter_context(tc.tile_pool(name="gb", bufs=2))
            smp = ctxA.enter_context(tc.tile_pool(name="small", bufs=16))
            psT = ctxA.enter_context(tc.tile_pool(name="psT", bufs=1, space="PSUM"))
            psY = ctxA.enter_context(tc.tile_pool(name="psY", bufs=NCH, space="PSUM"))

            ident = consts.tile([128, 128], F32)
            nc.sync.dma_start(ident[:], ident_d[:])
            eps_t = consts.tile([128, 1], F32)
            nc.sync.dma_start(eps_t[:], eps_d[:])

            h_cur = []
            for t in range(TT):
                st = state.tile([128, H], F32, name=f"hinit{t}", tag="state")
                nc.sync.dma_start(st[:], h0[t])
                h_cur.append(st)

            def transpose_cast(src_f32, scale_imm):
                """h [128tok, H] f32 -> hT [128feat-in-blk, (kt,128tok)] bf16 * scale."""
                pT = psT.tile([128, H], F32, tag="psT")
                for kt in range(KT):
                    nc.tensor.transpose(
                        pT[:, kt * 128 : (kt + 1) * 128],
                        src_f32[:, kt * 128 : (kt + 1) * 128],
                        ident[:],
                    )
                dst = hTp.tile([128, H], BF16, tag="hT")
                nc.scalar.activation(dst[:], pT[:], AF.Copy, scale=float(scale_imm))
                return dst

            def ln_finish(affine_src, S_ap, SS_ap, g_t, b_t, name):
                S = smp.tile([128, 1], F32, tag="s0", name=f"S{name}")
                SS = smp.tile([128, 1], F32, tag="s1", name=f"SS{name}")
                nc.vector.tensor_reduce(S[:], S_ap, axis=AX.X, op=OP.add)
                nc.vector.tensor_reduce(SS[:], SS_ap, axis=AX.X, op=OP.add)
                negmean = smp.tile([128, 1], F32, tag="s2", name=f"nm{name}")
                nc.vector.tensor_scalar_mul(negmean[:], S[:], -1.0 / H)
                msq = smp.tile([128, 1], F32, tag="s3", name=f"msq{name}")
                nc.vector.tensor_scalar_mul(msq[:], SS[:], 1.0 / H)
                var = smp.tile([128, 1], F32, tag="s4", name=f"var{name}")
                nc.vector.tensor_tensor(var[:], negmean[:], negmean[:], OP.mult)
                nc.vector.tensor_tensor(var[:], msq[:], var[:], OP.subtract)
                std = smp.tile([128, 1], F32, tag="s5", name=f"std{name}")
                nc.scalar.activation(std[:], var[:], AF.Sqrt, bias=eps_t[:])
                rstd = smp.tile([128, 1], F32, tag="s6", name=f"rstd{name}")
                nc.vector.reciprocal(rstd[:], std[:])
                hn = state.tile([128, H], F32, tag="state", name=f"h{name}")
                nc.vector.tensor_scalar(
                    hn[:], affine_src[:], negmean[:], rstd[:], OP.add, OP.mult
                )
                if g_t is not None:
                    nc.vector.tensor_tensor(hn[:], hn[:], g_t[:], OP.mult)
                    nc.vector.tensor_tensor(hn[:], hn[:], b_t[:], OP.add)
                return hn

            use_gb = not cfg.get("BISECT_NOBCAST", False)
            for l in range(L if cfg.get("DO_LAYERS", True) else 0):
                w_half = []
                for hf in range(2):
                    wt = wp.tile([128, KH, H], BF16, tag="w", name=f"w{l}_{hf}")
                    nc.sync.dma_start(
                        wt[:],
                        w_[l, hf * KH : (hf + 1) * KH].rearrange("k p o -> p k o"),
                    )
                    w_half.append(wt)
                g_t = b_t = bias_t = None
                if use_gb:
                    g_t = gbp.tile([128, H], BF16, tag="g", name=f"g{l}")
                    nc.sync.dma_start(g_t[:], lng[l][None, :].to_broadcast((128, H)))
                    b_t = gbp.tile([128, H], BF16, tag="b", name=f"b{l}")
                    nc.sync.dma_start(b_t[:], lnb[l][None, :].to_broadcast((128, H)))
                    bias_t = gbp.tile([128, H], BF16, tag="bias", name=f"bias{l}")
                    nc.sync.dma_start(
                        bias_t[:], lbias[l][None, :].to_broadcast((128, H))
                    )

                for t in range(TT):
                    hTt = transpose_cast(h_cur[t], scales[l])
                    ps = []
                    for i in range(NCH):
                        p = psY.tile([128, CH], F32, tag="psY", name=f"ps{l}_{t}_{i}")
                        ps.append(p)
                    for kt in range(KT):
                        wt = w_half[kt // KH]
                        for i in range(NCH):
                            nc.tensor.matmul(
                                ps[i][:],
                                lhsT=hTt[:, kt * 128 : (kt + 1) * 128],
                                rhs=wt[:, kt % KH, i * CH : (i + 1) * CH],
                                start=(kt == 0),
                                stop=(kt == KT - 1),
                            )
                    z = zpool.tile([128, H], F32, tag="z", name=f"z{l}_{t}")
                    sums = smp.tile(
                        [128, 1 + NCH], F32, tag="sums", name=f"sm{l}_{t}"
                    )
                    resid = h_cur[t]
                    if use_gb:
                        hb = zpool.tile([128, H], F32, tag="hb", name=f"hb{l}_{t}")
                        nc.vector.tensor_tensor(
                            hb[:], h_cur[t][:], bias_t[:], OP.add
                        )
                        resid = hb
                    for i in range(NCH):
                        nc.vector.tensor_add(
                            z[:, i * CH : (i + 1) * CH],
                            ps[i][:],
                            resid[:, i * CH : (i + 1) * CH],
                        )
                    nc.vector.tensor_reduce(
                        sums[:, 0:1], z[:], axis=AX.X, op=OP.add
                    )
                    for i in range(NCH):
                        nc.scalar.activation(
                            ps[i][:],
                            z[:, i * CH : (i + 1) * CH],
                            AF.Square,
                            accum_out=sums[:, 1 + i : 2 + i],
                        )
                    h_cur[t] = ln_finish(
                        z, sums[:, 0:1], sums[:, 1 : 1 + NCH], g_t, b_t,
                        f"{l}_{t}",
                    )

            # final LN + head-input transposes
            fg = fb = None
            if use_gb:
                fg = gbp.tile([128, H], BF16, tag="g", name="gfin")
                nc.sync.dma_start(fg[:], fing[None, :].to_broadcast((128, H)))
                fb = gbp.tile([128, H], BF16, tag="b", name="bfin")
                nc.sync.dma_start(fb[:], finb[None, :].to_broadcast((128, H)))
            for t in range(TT):
                h8 = h_cur[t]
                sums = smp.tile(
                    [128, 1 + NCH], F32, tag="sums", name=f"smfin{t}"
                )
                nc.vector.tensor_reduce(sums[:, 0:1], h8[:], axis=AX.X, op=OP.add)
                for i in range(NCH):
                    dump = psY.tile([128, CH], F32, tag="psY", name=f"dmp{t}_{i}")
                    nc.scalar.activation(
                        dump[:],
                        h8[:, i * CH : (i + 1) * CH],
                        AF.Square,
                        accum_out=sums[:, 1 + i : 2 + i],
                    )
                hfin = ln_finish(h8, sums[:, 0:1], sums[:, 1 : 1 + NCH], fg, fb, f"fin{t}")
                hTt = transpose_cast(hfin, head_scale)
                st_i = nc.sync.dma_start(
                    hT_loc[:, t], hTt[:].rearrange("p (k u) -> p k u", k=KT)
                )
                hT_store_insts.append(st_i)

        do_head = cfg.get("DO_HEAD", True)
        cc = None
        if do_head:
            cc = nc.gpsimd.collective_compute(
                "AllGather",
                OP.bypass,
                replica_groups=[list(range(NC))],
                ins=[hT_loc[:]],
                outs=[hT_all[:]],
            )
            for st_i in hT_store_insts:
                tile.add_dep_helper(
                    cc.ins, st_i.ins, sync=True, reason="gather waits on hT stores"
                )

        if do_head:
            with ExitStack() as ctxB:
                hTap = ctxB.enter_context(tc.tile_pool(name="hTall", bufs=1))
                wqp = ctxB.enter_context(tc.tile_pool(name="wq", bufs=2))
                outp = ctxB.enter_context(tc.tile_pool(name="outstg", bufs=4))
                psH = ctxB.enter_context(
                    tc.tile_pool(name="psH", bufs=6, space="PSUM")
                )

                hTall = hTap.tile([128, NC, TT, KT, 128], BF16)
                for c0 in range(0, NC, 2):
                    ld = nc.sync.dma_start(
                        hTall[:, c0 : c0 + 2],
                        hT_all[c0 : c0 + 2].rearrange("c p t k u -> p c t k u"),
                    )
                    tile.add_dep_helper(
                        ld.ins, cc.ins, sync=True, reason="hTall load waits on gather"
                    )
                for q in range(NQ):
                    wq = wqp.tile([128, KT, QV], BF16, tag="wq", name=f"wq{q}")
                    nc.sync.dma_start(
                        wq[:],
                        hw_[:, :, q * QV : (q + 1) * QV].rearrange("k p v -> p k v"),
                    )
                    for c in range(NC):
                        for t in range(TT):
                            pss = [
                                psH.tile(
                                    [128, NV], F32, tag="psH",
                                    name=f"ph{q}_{c}_{t}_{v}",
                                )
                                for v in range(NVQ)
                            ]
                            for kt in range(KT):
                                for vi in range(NVQ):
                                    nc.tensor.matmul(
                                        pss[vi][:],
                                        lhsT=hTall[:, c, t, kt, :],
                                        rhs=wq[:, kt, vi * NV : (vi + 1) * NV],
                                        start=(kt == 0),
                                        stop=(kt == KT - 1),
                                        skip_group_check=True,
                                    )
                            row0 = (c * TT + t) * 128
                            o_t = outp.tile(
                                [128, QV], F32, tag="ostg", name=f"o{q}_{c}_{t}"
                            )
                            for vi in range(NVQ):
                                nc.scalar.copy(
                                    o_t[:, vi * NV : (vi + 1) * NV], pss[vi][:]
                                )
                            nc.sync.dma_start(
                                out[row0 : row0 + 128, q * QV : (q + 1) * QV], o_t[:]
                            )

    return nc


def _ternary(wmat):
    """Exact {-1,0,1} ternary tensor + fp32 scale, matching the reference."""
    w = np.asarray(wmat, dtype=np.float32)
    s = np.mean(np.abs(w), dtype=np.float32)
    t = np.clip(np.rint(w / (s + np.float32(1e-8))), -1.0, 1.0).astype(np.float32)
    return t, float(s)


_NC_CACHE = {}
_LAST_RESULTS = None


def kernel(**inputs):
    global _LAST_RESULTS
    cfg = CFG_FULL
    L, H, NTOK, NC, TT, VS = (
        cfg["L"], cfg["H"], cfg["NTOK"], cfg["NC"], cfg["TT"], cfg["VS"],
    )
    KT = H // 128
    TPC = TT * 128  # tokens per core
    BF = ml_dtypes.bfloat16

    ids = np.asarray(inputs["input_ids"]).astype(np.int64).reshape(-1)
    embed = np.asarray(inputs["embed"], dtype=np.float32)
    layer_w = np.asarray(inputs["layer_w"], dtype=np.float32)
    layer_b = np.asarray(inputs["layer_b"], dtype=np.float32)
    ln_g = np.asarray(inputs["ln_g"], dtype=np.float32)
    ln_b = np.asarray(inputs["ln_b"], dtype=np.float32)
    final_g = np.asarray(inputs["final_g"], dtype=np.float32)
    final_b = np.asarray(inputs["final_b"], dtype=np.float32)
    head_w = np.asarray(inputs["head_w"], dtype=np.float32)

    h0_full = embed[ids]  # [NTOK, H] fp32

    scales = []
    wT = np.empty([L, KT, 128, H], dtype=BF)
    for l in range(L):
        t, s = _ternary(layer_w[l])
        scales.append(s)
        wT[l] = np.ascontiguousarray(t.T).reshape(KT, 128, H).astype(BF)
    th, head_scale = _ternary(head_w)
    headT = np.ascontiguousarray(th.T).astype(BF)  # [H, V]

    key = (id(cfg), tuple(scales), head_scale)
    if key not in _NC_CACHE:
        _NC_CACHE.clear()
        nc = build_nc(cfg, scales, head_scale)
        # Bacc.finalize runs the TRN2 legalization passes (1-wait-per-
        # instruction event-semaphore split, matmul->ldweights wait motion,
        # register allocation). The PJRT exec path serializes nc as-is.
        nc.finalize()
        _NC_CACHE[key] = nc
    nc = _NC_CACHE[key]

    common = {
        "w": wT,
        "lng": ln_g.astype(BF),
        "lnb": ln_b.astype(BF),
        "lbias": layer_b.astype(BF),
        "fing": final_g.astype(BF),
        "finb": final_b.astype(BF),
        "ident": np.eye(128, dtype=np.float32),
        "eps": np.full((128, 1), EPS, np.float32),
    }
    in_maps = []
    for c in range(NC):
        in_maps.append(
            dict(
                common,
                h0=np.ascontiguousarray(
                    h0_full[c * TPC : (c + 1) * TPC].reshape(TT, 128, H)
                ),
                hw=np.ascontiguousarray(
                    headT[:, c * VS : (c + 1) * VS].reshape(KT, 128, VS)
                ),
            )
        )

    trace = bool(int(os.environ.get("TRIKERNEL_TRACE", "0")))
    res = run_bass_kernel_spmd(nc, in_maps, core_ids=list(range(NC)), trace=trace)
    _LAST_RESULTS = res

    full = np.concatenate(
        [np.asarray(res.results[c]["out"]) for c in range(NC)], axis=1
    )  # [NTOK, V]
    return full.reshape(2, 1024, 32000).astype(np.float32)



# revision 2
# speedup vs baseline: 1.2988x; 1.2988x over previous
"""Trainium2 Bass kernel: 8-layer ternary (BitNet-1.58) dense transformer.

Model (per reference):
    h = embed[input_ids]                                  # (B=2, S=1024, H=2048)
    8x: y = h @ ternary(W_l)^T + b_l ; h = LN(y + h)*g+b  # H=2048
    h = LN(h)*final_g + final_b
    logits = h @ ternary(head_W)^T                        # (B, S, V=32000)

Sharding over 8 NeuronCores (fully local, no collectives):
  - Layers: data-parallel over the 2048 tokens (256 tokens/core). Each core
    streams the full (bf16 ternary) layer weights.
  - Head: ALSO data-parallel over tokens: each core computes its own 256
    tokens x the full 32000-entry vocab. The ternary head weights are sent
    as exact {-1,0,+1} fp8(e4m3) and streamed chunk-by-chunk (65.5 MB/core),
    overlapped with compute. No AllGather.

Ternary weights are exact {-1,0,+1} tensors; the per-tensor fp32 scale is
folded into the activation transpose-copy (ScalarE) so matmul lhsT inputs
are bf16 while the residual/LN path stays fp32.
"""

import os
import sys

import numpy as np

try:
    import concourse.bass as bass
except ImportError:  # grading container should have it on sys.path already
    sys.path.insert(0, "/opt/trn_rl_repo")
    import concourse.bass as bass

import ml_dtypes
import concourse.mybir as mybir
import concourse.tile as tile
from concourse import bacc
from concourse.bass_utils import run_bass_kernel_spmd
from contextlib import ExitStack

F32 = mybir.dt.float32
BF16 = mybir.dt.bfloat16
FP8E4 = mybir.dt.float8e4
AX = mybir.AxisListType
OP = mybir.AluOpType
AF = mybir.ActivationFunctionType
EPS = 1e-5

# Full-size problem config (B=2, S=1024 -> 2048 tokens, 256/core).
CFG_FULL = dict(L=8, H=2048, NC=8, TT=2, V=32000, QV=500, CH=512)


def build_nc(cfg, scales, head_scale, triv_ln, fp8_w):
    L, H, NC, TT = cfg["L"], cfg["H"], cfg["NC"], cfg["TT"]
    V, QV, CH = cfg["V"], cfg["QV"], cfg["CH"]
    KT = H // 128
    KH = KT // 2  # k-tiles per layer-weight half
    NCH = H // CH
    NQ = V // QV
    assert V % QV == 0 and H % CH == 0
    WDT = FP8E4 if fp8_w else BF16

    nc = bacc.Bacc("TRN2", target_bir_lowering=False, debug=False, num_devices=NC)
    h0 = nc.declare_dram_parameter("h0", [TT, 128, H], F32, isOutput=False)
    w_ = nc.declare_dram_parameter("w", [L, KT, 128, H], WDT, isOutput=False)
    if not triv_ln:
        lng = nc.declare_dram_parameter("lng", [L, H], BF16, isOutput=False)
        lnb = nc.declare_dram_parameter("lnb", [L, H], BF16, isOutput=False)
        lbias = nc.declare_dram_parameter("lbias", [L, H], BF16, isOutput=False)
        fing = nc.declare_dram_parameter("fing", [H], BF16, isOutput=False)
        finb = nc.declare_dram_parameter("finb", [H], BF16, isOutput=False)
    hw_ = nc.declare_dram_parameter("hw", [NQ, 128, KT, QV], WDT, isOutput=False)
    ident_d = nc.declare_dram_parameter("ident", [128, 128], F32, isOutput=False)
    eps_d = nc.declare_dram_parameter("eps", [128, 1], F32, isOutput=False)
    out = nc.declare_dram_parameter("out", [TT * 128, V], F32, isOutput=True)

    with tile.TileContext(nc) as tc:
        with ExitStack() as ctx0:
            consts = ctx0.enter_context(tc.tile_pool(name="consts", bufs=1))
            state = ctx0.enter_context(tc.tile_pool(name="state", bufs=4))
            hTp = ctx0.enter_context(tc.tile_pool(name="hT", bufs=2))
            wqp = ctx0.enter_context(tc.tile_pool(name="wq", bufs=3))
            outp = ctx0.enter_context(tc.tile_pool(name="outstg", bufs=4))
            smp = ctx0.enter_context(tc.tile_pool(name="small", bufs=16))

            ident = consts.tile([128, 128], F32)
            nc.sync.dma_start(ident[:], ident_d[:])
            eps_t = consts.tile([128, 1], F32)
            nc.sync.dma_start(eps_t[:], eps_d[:])

            h_cur = []
            for t in range(TT):
                st = state.tile([128, H], F32, name=f"hinit{t}", tag="state")
                nc.sync.dma_start(st[:], h0[t])
                h_cur.append(st)

            hT_cur = [None] * TT  # bf16 pre-scaled transposed activations

            with ExitStack() as ctxA:
                zpool = ctxA.enter_context(tc.tile_pool(name="z", bufs=2))
                wp = ctxA.enter_context(tc.tile_pool(name="w", bufs=3))
                sqp = ctxA.enter_context(tc.tile_pool(name="sq", bufs=2))
                gbp = None
                if not triv_ln:
                    gbp = ctxA.enter_context(tc.tile_pool(name="gb", bufs=2))
                psT = ctxA.enter_context(
                    tc.tile_pool(name="psT", bufs=2, space="PSUM")
                )
                psY = ctxA.enter_context(
                    tc.tile_pool(name="psY", bufs=NCH, space="PSUM")
                )

                def transpose_cast(src_f32, scale_imm, name):
                    """h [128tok, H] f32 -> hT [128feat-in-blk, (kt,128tok)] bf16*s.

                    Split into 2 halves (2 PSUM banks each) so the scalar
                    copy of half 0 overlaps the PE transposes of half 1.
                    """
                    dst = hTp.tile([128, H], BF16, tag="hT", name=f"hT{name}")
                    for hf in range(2):
                        pT = psT.tile([128, H // 2], F32, tag="psT", name=f"pT{name}_{hf}")
                        for k in range(KT // 2):
                            kt = hf * (KT // 2) + k
                            nc.tensor.transpose(
                                pT[:, k * 128 : (k + 1) * 128],
                                src_f32[:, kt * 128 : (kt + 1) * 128],
                                ident[:],
                            )
                        nc.scalar.activation(
                            dst[:, hf * (H // 2) : (hf + 1) * (H // 2)],
                            pT[:],
                            AF.Copy,
                            scale=float(scale_imm),
                        )
                    return dst

                def ln_and_transpose(z, S_ap, SS_ap, g_t, b_t, next_scale, name):
                    """LayerNorm finish from precomputed sums; returns
                    (h_f32, hT_bf16_prescaled)."""
                    S = smp.tile([128, 1], F32, tag="s0", name=f"S{name}")
                    SS = smp.tile([128, 1], F32, tag="s1", name=f"SS{name}")
                    nc.vector.tensor_reduce(S[:], S_ap, axis=AX.X, op=OP.add)
                    nc.vector.tensor_reduce(SS[:], SS_ap, axis=AX.X, op=OP.add)
                    negmean = smp.tile([128, 1], F32, tag="s2", name=f"nm{name}")
                    nc.vector.tensor_scalar_mul(negmean[:], S[:], -1.0 / H)
                    msq = smp.tile([128, 1], F32, tag="s3", name=f"msq{name}")
                    nc.vector.tensor_scalar_mul(msq[:], SS[:], 1.0 / H)
                    var = smp.tile([128, 1], F32, tag="s4", name=f"var{name}")
                    nc.vector.tensor_tensor(var[:], negmean[:], negmean[:], OP.mult)
                    nc.vector.tensor_tensor(var[:], msq[:], var[:], OP.subtract)
                    std = smp.tile([128, 1], F32, tag="s5", name=f"std{name}")
                    nc.scalar.activation(std[:], var[:], AF.Sqrt, bias=eps_t[:])
                    rstd = smp.tile([128, 1], F32, tag="s6", name=f"rstd{name}")
                    nc.vector.reciprocal(rstd[:], std[:])
                    hn = state.tile([128, H], F32, tag="state", name=f"h{name}")
                    nc.vector.tensor_scalar(
                        hn[:], z[:], negmean[:], rstd[:], OP.add, OP.mult
                    )
                    if g_t is not None:
                        nc.vector.tensor_tensor(hn[:], hn[:], g_t[:], OP.mult)
                        nc.vector.tensor_tensor(hn[:], hn[:], b_t[:], OP.add)
                    hT = transpose_cast(hn, next_scale, name)
                    return hn, hT

                # prologue: transpose the embedding activations for layer 0
                for t in range(TT):
                    hT_cur[t] = transpose_cast(h_cur[t], scales[0], f"p{t}")

                for l in range(L):
                    w_half = []
                    for hf in range(2):
                        wt = wp.tile([128, KH, H], WDT, tag="w", name=f"w{l}_{hf}")
                        nc.sync.dma_start(
                            wt[:],
                            w_[l, hf * KH : (hf + 1) * KH].rearrange(
                                "k p o -> p k o"
                            ),
                        )
                        w_half.append(wt)
                    g_t = b_t = bias_t = None
                    if not triv_ln:
                        g_t = gbp.tile([128, H], BF16, tag="g", name=f"g{l}")
                        nc.sync.dma_start(
                            g_t[:], lng[l][None, :].to_broadcast((128, H))
                        )
                        b_t = gbp.tile([128, H], BF16, tag="b", name=f"b{l}")
                        nc.sync.dma_start(
                            b_t[:], lnb[l][None, :].to_broadcast((128, H))
                        )
                        bias_t = gbp.tile([128, H], BF16, tag="bias", name=f"bias{l}")
                        nc.sync.dma_start(
                            bias_t[:], lbias[l][None, :].to_broadcast((128, H))
                        )

                    next_scale = scales[l + 1] if l + 1 < L else head_scale
                    for t in range(TT):
                        hTt = hT_cur[t]
                        ps = [
                            psY.tile([128, CH], F32, tag="psY", name=f"ps{l}_{t}_{i}")
                            for i in range(NCH)
                        ]
                        # kt-major within chunk-halves: psums 0,1 complete
                        # (and start draining) while PE works on 2,3.
                        for half in range(2):
                            for kt in range(KT):
                                wt = w_half[kt // KH]
                                for i in (2 * half, 2 * half + 1):
                                    nc.tensor.matmul(
                                        ps[i][:],
                                        lhsT=hTt[:, kt * 128 : (kt + 1) * 128],
                                        rhs=wt[:, kt % KH, i * CH : (i + 1) * CH],
                                        start=(kt == 0),
                                        stop=(kt == KT - 1),
                                    )
                        z = zpool.tile([128, H], F32, tag="z", name=f"z{l}_{t}")
                        sums = smp.tile(
                            [128, 1 + NCH], F32, tag="sums", name=f"sm{l}_{t}"
                        )
                        resid = h_cur[t]
                        if not triv_ln:
                            hb = zpool.tile([128, H], F32, tag="hb", name=f"hb{l}_{t}")
                            nc.vector.tensor_tensor(
                                hb[:], h_cur[t][:], bias_t[:], OP.add
                            )
                            resid = hb
                        for i in range(NCH):
                            nc.vector.tensor_add(
                                z[:, i * CH : (i + 1) * CH],
                                ps[i][:],
                                resid[:, i * CH : (i + 1) * CH],
                            )
                        nc.vector.tensor_reduce(
                            sums[:, 0:1], z[:], axis=AX.X, op=OP.add
                        )
                        for i in range(NCH):
                            # Square's bulk output is scratch; send it to a
                            # small SBUF tile (NOT psum) so the matmul banks
                            # free up as soon as the z adds have read them.
                            sq = sqp.tile(
                                [128, CH], BF16, tag="sq", name=f"sq{l}_{t}_{i}"
                            )
                            nc.scalar.activation(
                                sq[:],
                                z[:, i * CH : (i + 1) * CH],
                                AF.Square,
                                accum_out=sums[:, 1 + i : 2 + i],
                            )
                        h_cur[t], hT_cur[t] = ln_and_transpose(
                            z, sums[:, 0:1], sums[:, 1 : 1 + NCH], g_t, b_t,
                            next_scale, f"{l}_{t}",
                        )

                # final LN (+ fold-in of head ternary scale via transpose)
                fg = fb = None
                if not triv_ln:
                    fg = gbp.tile([128, H], BF16, tag="g", name="gfin")
                    nc.sync.dma_start(fg[:], fing[None, :].to_broadcast((128, H)))
                    fb = gbp.tile([128, H], BF16, tag="b", name="bfin")
                    nc.sync.dma_start(fb[:], finb[None, :].to_broadcast((128, H)))
                for t in range(TT):
                    h8 = h_cur[t]
                    sums = smp.tile(
                        [128, 1 + NCH], F32, tag="sums", name=f"smfin{t}"
                    )
                    nc.vector.tensor_reduce(
                        sums[:, 0:1], h8[:], axis=AX.X, op=OP.add
                    )
                    for i in range(NCH):
                        sq = sqp.tile(
                            [128, CH], BF16, tag="sq", name=f"sqf{t}_{i}"
                        )
                        nc.scalar.activation(
                            sq[:],
                            h8[:, i * CH : (i + 1) * CH],
                            AF.Square,
                            accum_out=sums[:, 1 + i : 2 + i],
                        )
                    h_cur[t], hT_cur[t] = ln_and_transpose(
                        h8, sums[:, 0:1], sums[:, 1 : 1 + NCH], fg, fb,
                        head_scale, f"fin{t}",
                    )

            # ---- head: own 256 tokens x full vocab, streamed fp8 weights ----
            with ExitStack() as ctxB:
                psH = ctxB.enter_context(
                    tc.tile_pool(name="psH", bufs=6, space="PSUM")
                )
                for q in range(NQ):
                    wq = wqp.tile([128, KT, QV], WDT, tag="wq", name=f"wq{q}")
                    nc.sync.dma_start(wq[:], hw_[q])
                    for t in range(TT):
                        ph = psH.tile([128, QV], F32, tag="psH", name=f"ph{q}_{t}")
                        for kt in range(KT):
                            nc.tensor.matmul(
                                ph[:],
                                lhsT=hT_cur[t][:, kt * 128 : (kt + 1) * 128],
                                rhs=wq[:, kt, :],
                                start=(kt == 0),
                                stop=(kt == KT - 1),
                            )
                        o_t = outp.tile([128, QV], F32, tag="ostg", name=f"o{q}_{t}")
                        nc.scalar.copy(o_t[:], ph[:])
                        nc.sync.dma_start(
                            out[t * 128 : (t + 1) * 128, q * QV : (q + 1) * QV],
                            o_t[:],
                        )

    return nc


def _ternary(wmat):
    """Exact {-1,0,1} ternary tensor + fp32 scale, matching the reference."""
    w = np.asarray(wmat, dtype=np.float32)
    s = np.mean(np.abs(w), dtype=np.float32)
    t = np.clip(np.rint(w / (s + np.float32(1e-8))), -1.0, 1.0).astype(np.float32)
    return t, float(s)


_NC_CACHE = {}
_LAST_RESULTS = None


def kernel(**inputs):
    global _LAST_RESULTS
    cfg = CFG_FULL
    L, H, NC, TT, V, QV = (
        cfg["L"], cfg["H"], cfg["NC"], cfg["TT"], cfg["V"], cfg["QV"],
    )
    KT = H // 128
    NQ = V // QV
    TPC = TT * 128  # tokens per core
    BF = ml_dtypes.bfloat16
    F8 = ml_dtypes.float8_e4m3fn
    fp8_w = not bool(int(os.environ.get("TRIKERNEL_BF16_W", "0")))
    WNP = F8 if fp8_w else BF

    ids = np.asarray(inputs["input_ids"]).astype(np.int64).reshape(-1)
    embed = np.asarray(inputs["embed"], dtype=np.float32)
    layer_w = np.asarray(inputs["layer_w"], dtype=np.float32)
    layer_b = np.asarray(inputs["layer_b"], dtype=np.float32)
    ln_g = np.asarray(inputs["ln_g"], dtype=np.float32)
    ln_b = np.asarray(inputs["ln_b"], dtype=np.float32)
    final_g = np.asarray(inputs["final_g"], dtype=np.float32)
    final_b = np.asarray(inputs["final_b"], dtype=np.float32)
    head_w = np.asarray(inputs["head_w"], dtype=np.float32)

    # trivial-affine specialization: the LN scale/shift and layer bias are
    # identity in this model instance; skip them on-chip when so.
    triv_ln = bool(
        np.all(ln_g == 1.0) and np.all(ln_b == 0.0) and np.all(layer_b == 0.0)
        and np.all(final_g == 1.0) and np.all(final_b == 0.0)
    )

    h0_full = embed[ids]  # [NTOK, H] fp32

    scales = []
    wT = np.empty([L, KT, 128, H], dtype=WNP)
    for l in range(L):
        t, s = _ternary(layer_w[l])
        scales.append(s)
        wT[l] = np.ascontiguousarray(t.T).reshape(KT, 128, H).astype(WNP)
    th, head_scale = _ternary(head_w)
    # head weights, laid out so each [128, KT, QV] vocab chunk is a single
    # contiguous 8KB-per-partition DMA: hw8[q, p, kt, v]
    hw8 = np.ascontiguousarray(
        th.T.reshape(KT, 128, NQ, QV).transpose(2, 1, 0, 3)
    ).astype(WNP)

    key = (id(cfg), tuple(scales), head_scale, triv_ln, fp8_w)
    if key not in _NC_CACHE:
        _NC_CACHE.clear()
        nc = build_nc(cfg, scales, head_scale, triv_ln, fp8_w)
        # Bacc.finalize runs the TRN2 legalization passes (1-wait-per-
        # instruction event-semaphore split, matmul->ldweights wait motion,
        # register allocation). The PJRT exec path serializes nc as-is.
        nc.finalize()
        _NC_CACHE[key] = nc
    nc = _NC_CACHE[key]

    common = {
        "w": wT,
        "hw": hw8,
        "ident": np.eye(128, dtype=np.float32),
        "eps": np.full((128, 1), EPS, np.float32),
    }
    if not triv_ln:
        common.update(
            lng=ln_g.astype(BF),
            lnb=ln_b.astype(BF),
            lbias=layer_b.astype(BF),
            fing=final_g.astype(BF),
            finb=final_b.astype(BF),
        )
    in_maps = []
    for c in range(NC):
        in_maps.append(
            dict(
                common,
                h0=np.ascontiguousarray(
                    h0_full[c * TPC : (c + 1) * TPC].reshape(TT, 128, H)
                ),
            )
        )

    trace = bool(int(os.environ.get("TRIKERNEL_TRACE", "0")))
    res = run_bass_kernel_spmd(nc, in_maps, core_ids=list(range(NC)), trace=trace)
    _LAST_RESULTS = res

    full = np.concatenate(
        [np.asarray(res.results[c]["out"]) for c in range(NC)], axis=0
    )  # [NTOK, V]
    return full.reshape(2, 1024, 32000).astype(np.float32)
